# revision 1
# baseline (speedup 1.0000x reference)
"""Trainium2 Bass kernel for CombinedVectorField (CFG vector field + exact
Jacobian-trace divergence).

Math: with u = tanh(x@W1x + h@W1h + b1'), b1' = b1 + t*W1[256],
  v(x,h)  = u @ W2 + b2
  div(x,h)= sum_k (1-u_k^2) c_k = d0 - (u*u) @ c,   c_k = sum_i W1x[i,k] W2[k,i]
Output = concat[(1-gs)*v_null + gs*v_h, (1-gs)*div_null + gs*div_h].

Sharding: pure data parallel — each of the 8 cores takes 512 batch rows
(both guidance branches), weights replicated. All tensors are kept
feature-major (transposed) on device so every matmul contracts over the
partition dim; host does the transposes/reshapes only.
"""
import sys

sys.path.insert(0, "/opt/trn_rl_repo")

import ml_dtypes
import numpy as np

import concourse.bass as bass
import concourse.tile as tile
from concourse import bacc, mybir
from concourse.bass_utils import run_bass_kernel_spmd
from concourse.vector_clock import ScopedClock


class _TrimTileContext(tile.TileContext):
    """TileContext with the final all-engine barrier dropped from the
    teardown and the mid barrier reduced to sem-only (no per-engine
    drains). The head drain still waits for every semaphore (incl.
    output-DMA completion) and semaphores are still cleared for the next
    execution; only the trailing barrier (nothing executes after it) is
    elided."""

    def _drain_and_barrier(self, tick_clock, wait_clock):
        drain_inst = self.nc.sync.drain()
        wait_clock.add_sem_waits(
            drain_inst.ins, ScopedClock({None: tick_clock.global_clock})
        )
        self.nc.all_engine_barrier(sem_only=True)
        popped = self.nc._tile_sem_poison_stack.pop()
        assert popped is self._sem_poison
        self.nc.clear_and_free_semaphores(list(self.sems.allocated().values()))


class _FastBacc(bacc.Bacc):
    """Bacc whose constructor-time all-engine barrier (after the const-tile
    memsets) is sem-only — the per-engine drains there cost ~1us of kernel
    head time and order nothing we rely on beyond the memsets, which the
    event-semaphore barrier already orders."""

    def all_engine_barrier(self, *, sem_only: bool = False):
        super().all_engine_barrier(sem_only=True)

F32 = mybir.dt.float32
BF16 = mybir.dt.bfloat16
AF = mybir.ActivationFunctionType
ALU = mybir.AluOpType

N_CORES = 8
B = 4096
DIM_X = 128
DIM_H = 128
HIDDEN = 512
R = B // N_CORES          # rows per core
NCH = HIDDEN // 128       # hidden chunks
W2W = NCH * DIM_X + NCH   # w2 chunks + cmat columns

_NC_CACHE = None


def _build():
    nc = _FastBacc("TRN2", target_bir_lowering=False, debug=False,
                   enable_asserts=False, monotonic_sem_count=0)

    # four bf16 input blobs, alternating over the two HWDGE rings so the
    # first-matmul gate (A1 + B1) is as small as possible; the w2 blob (B2)
    # is only needed once the first tanh completes.
    #   A1 = [xT | w1x]   A2 = [w1h]   B1 = [hT | hnT]
    #   B2 = [gs*w2r | (1-gs)*w2r | -gs*cmat | -(1-gs)*cmat]
    # (guidance-scale combine folded into the weights on the host, so PSUM
    #  accumulates the already-combined v and div directly)
    inA1 = nc.dram_tensor("inA1", [128, R + HIDDEN], BF16, kind="ExternalInput")
    inA2 = nc.dram_tensor("inA2", [128, HIDDEN], BF16, kind="ExternalInput")
    inB1 = nc.dram_tensor("inB1", [128, 2 * R], BF16, kind="ExternalInput")
    inB2 = nc.dram_tensor("inB2", [128, 2 * W2W], BF16, kind="ExternalInput")
    # aux cols: 0-3 b1' chunks, 4 b2, 5 d0
    aux = nc.dram_tensor("aux", [128, 6], F32, kind="ExternalInput")

    VO = nc.dram_tensor("VO", [DIM_X, R], F32, kind="ExternalOutput")
    DO = nc.dram_tensor("DO", [1, R], F32, kind="ExternalOutput")

    with _TrimTileContext(nc) as tc:
        with tc.tile_pool(name="cst", bufs=1) as cst, \
             tc.tile_pool(name="act", bufs=3) as actp, \
             tc.tile_pool(name="out", bufs=1) as outp, \
             tc.tile_pool(name="psa", bufs=6, space="PSUM") as psa, \
             tc.tile_pool(name="psv", bufs=1, space="PSUM") as psv:
            # PE prewarm: dummy f32 matmuls on a zeroed tile keep the PE-HAM
            # activity window busy during the input DMAs, so real matmuls run
            # at 2.4 GHz instead of 1.2 GHz.
            wrm = cst.tile([128, 256], F32)
            nc.gpsimd.memset(wrm[:], 0.0)
            pwarm = psa.tile([128, R], F32, tag="a")
            for _ in range(5):
                nc.tensor.matmul(pwarm[:, 0:256], wrm[:, 0:128], wrm[:],
                                 start=True, stop=True, skip_group_check=True)

            # scalar ring issues first (sync's first DMA waits on a drain),
            # so the first-matmul gate (A1) goes there
            a1t = cst.tile([128, R + HIDDEN], BF16)
            nc.scalar.dma_start(out=a1t[:], in_=inA1[:])
            a2t = cst.tile([128, HIDDEN], BF16)
            nc.sync.dma_start(out=a2t[:], in_=inA2[:])
            b1t = cst.tile([128, 2 * R], BF16)
            nc.sync.dma_start(out=b1t[:], in_=inB1[:])
            b2t = cst.tile([128, 2 * W2W], BF16)
            nc.gpsimd.dma_start(out=b2t[:], in_=inB2[:])
            auxt = cst.tile([128, 6], F32)
            nc.gpsimd.dma_start(out=auxt[:], in_=aux[:])

            xt = a1t[:, 0:R]
            w1x = a1t[:, R:R + HIDDEN]
            w1h = a2t[:]
            hst = b1t[:]
            w2b = [b2t[:, br * NCH * DIM_X:(br + 1) * NCH * DIM_X] for br in range(2)]
            cmb = [b2t[:, 2 * NCH * DIM_X + br * NCH:2 * NCH * DIM_X + (br + 1) * NCH]
                   for br in range(2)]

            # both branches accumulate into the same banks (weights pre-scaled
            # by gs/(1-gs), so the sum IS the guidance-combined result)
            pv = psv.tile([128, R], F32)
            pd = psv.tile([1, R], F32)

            # per-(chunk, branch) pieces: finer ACT/PSUM granularity keeps the
            # PE from stalling at chunk boundaries (rotating 1-bank a-tiles)
            for c in range(NCH):
                cs = bass.ts(c, 128)
                for br in range(2):
                    first = c == 0 and br == 0
                    last = c == NCH - 1 and br == 1
                    bs = bass.ts(br, R)            # branch slice in hst
                    a = psa.tile([128, R], F32, tag="a")
                    nc.tensor.matmul(a[:], w1x[:, cs], xt[:], start=True, stop=False)
                    nc.tensor.matmul(a[:], w1h[:, cs], hst[:, bs], start=False, stop=True)

                    u = actp.tile([128, R], BF16, tag="u")
                    nc.scalar.activation(u[:], a[:], AF.Tanh, bias=auxt[:, c:c + 1], scale=1.0)
                    u2 = actp.tile([128, R], BF16, tag="u2")
                    nc.vector.tensor_tensor(u2[:], u[:], u[:], op=ALU.mult)

                    nc.tensor.matmul(pv[:], w2b[br][:, cs], u[:], start=first, stop=last)
                    nc.tensor.matmul(pd[0:1, :], cmb[br][:, c:c + 1], u2[:], start=first, stop=last)

            # weights pre-scaled by gs/(1-gs)/-gs/-(1-gs): the PSUM sums ARE the
            # guidance-combined results; just add the bias terms. vout on ACT
            # and dout on DVE so the two PSUM->SBUF moves run in parallel.
            vout = outp.tile([128, R], F32)
            nc.scalar.activation(vout[:], pv[:], AF.Identity, bias=auxt[:, 4:5], scale=1.0)
            dout = outp.tile([1, R], F32)
            nc.vector.tensor_scalar(dout[:], pd[0:1, :], auxt[0:1, 5:6], None, op0=ALU.add)

            nc.sync.dma_start(out=VO[:], in_=vout[:])
            nc.scalar.dma_start(out=DO[:], in_=dout[:])
    nc.compile()
    return nc


def _get_nc():
    global _NC_CACHE
    if _NC_CACHE is None:
        _NC_CACHE = _build()
    return _NC_CACHE


def _prep_in_maps(state, h, h_null, t, guidance_scale, W1, b1, W2, b2):
    f32 = np.float32
    bf = ml_dtypes.bfloat16
    xTf = state[:, :DIM_X].T.astype(bf)                            # (128, B)
    hTf = h.T.astype(bf)
    hnTf = h_null.T.astype(bf)
    w1f = np.concatenate([W1[:DIM_X], W1[DIM_X:DIM_X + DIM_H]], axis=1).astype(bf)
    b1p = (b1.astype(f32) + t.astype(f32)[0] * W1[DIM_X + DIM_H].astype(f32))
    w2r = W2.astype(f32).reshape(NCH, 128, DIM_X).transpose(1, 0, 2).reshape(128, NCH * DIM_X)
    cvec = (W1[:DIM_X].astype(np.float64) * W2.astype(np.float64).T).sum(0)  # (512,)
    d0 = cvec.sum()
    cmatf = cvec.reshape(NCH, 128).T.astype(f32)                   # (128, NCH)
    gs = float(guidance_scale.astype(f32)[0])
    w2cf = np.concatenate([gs * w2r, (1.0 - gs) * w2r,
                           -gs * cmatf, -(1.0 - gs) * cmatf], axis=1).astype(bf)

    auxf = np.zeros((128, 6), f32)
    auxf[:, 0:4] = b1p.reshape(NCH, 128).T
    auxf[:, 4] = b2.astype(f32)
    auxf[:, 5] = d0

    w1xa = np.ascontiguousarray(w1f[:, :HIDDEN])
    w1ha = np.ascontiguousarray(w1f[:, HIDDEN:])
    in_maps = []
    for i in range(N_CORES):
        sl = slice(i * R, (i + 1) * R)
        in_maps.append({
            "inA1": np.ascontiguousarray(
                np.concatenate([xTf[:, sl], w1xa], axis=1)),
            "inA2": w1ha,
            "inB1": np.ascontiguousarray(
                np.concatenate([hTf[:, sl], hnTf[:, sl]], axis=1)),
            "inB2": w2cf,
            "aux": auxf,
        })
    return in_maps


def kernel(state, h, h_null, t, guidance_scale, W1, b1, W2, b2, _trace=False):
    nc = _get_nc()
    in_maps = _prep_in_maps(state, h, h_null, t, guidance_scale, W1, b1, W2, b2)
    res = run_bass_kernel_spmd(nc, in_maps, list(range(N_CORES)), trace=_trace)
    out = np.empty((B, DIM_X + 1), np.float32)
    for i in range(N_CORES):
        sl = slice(i * R, (i + 1) * R)
        out[sl, :DIM_X] = res.results[i]["VO"].T
        out[sl, DIM_X] = res.results[i]["DO"][0]
    if _trace:
        return out, res
    return out



# revision 2
# speedup vs baseline: 1.0445x; 1.0445x over previous
"""Trainium2 Bass kernel for CombinedVectorField (CFG vector field + exact
Jacobian-trace divergence).

Math: with u = tanh(x@W1x + h@W1h + b1'), b1' = b1 + t*W1[256],
  v(x,h)  = u @ W2 + b2
  div(x,h)= sum_k (1-u_k^2) c_k = d0 - (u*u) @ c,   c_k = sum_i W1x[i,k] W2[k,i]
Output = concat[(1-gs)*v_null + gs*v_h, (1-gs)*div_null + gs*div_h].

Sharding: pure data parallel - each of the 8 cores takes 512 batch rows
(both guidance branches), weights replicated. Feature-major layouts so
every matmul contracts over the partition dim.

Schedule: activations ship in two row-halves so L1 matmuls start before
all input bytes land; per hidden chunk the four (branch x half) L1
outputs share one [128,1024] PSUM group (single start=True per bank,
partial-region writes ride the pending-zero semantics) so tanh runs as
one [128,1024] ACTIVATE with a per-partition bias. Outputs leave as
bf16 (host upcasts) with VO on the sync HWDGE ring and DO on scalar.
"""
import sys

sys.path.insert(0, "/opt/trn_rl_repo")

import ml_dtypes
import numpy as np

import concourse.bass as bass
import concourse.tile as tile
from concourse import bacc, mybir
from concourse.bass_utils import run_bass_kernel_spmd
from concourse.vector_clock import ScopedClock


class _TrimTileContext(tile.TileContext):
    """TileContext with the final all-engine barrier dropped from the
    teardown and the mid barrier reduced to sem-only (no per-engine
    drains). The head drain still waits for every semaphore (incl.
    output-DMA completion) and semaphores are still cleared for the next
    execution; only the trailing barrier (nothing executes after it) is
    elided."""

    def _drain_and_barrier(self, tick_clock, wait_clock):
        drain_inst = self.nc.sync.drain()
        wait_clock.add_sem_waits(
            drain_inst.ins, ScopedClock({None: tick_clock.global_clock})
        )
        self.nc.all_engine_barrier(sem_only=True)
        popped = self.nc._tile_sem_poison_stack.pop()
        assert popped is self._sem_poison
        self.nc.clear_and_free_semaphores(list(self.sems.allocated().values()))


class _FastBacc(bacc.Bacc):
    """Bacc whose constructor-time all-engine barrier (after the const-tile
    memsets) is sem-only - the per-engine drains there cost ~1us of kernel
    head time and order nothing we rely on beyond the memsets, which the
    event-semaphore barrier already orders."""

    def all_engine_barrier(self, *, sem_only: bool = False):
        super().all_engine_barrier(sem_only=True)

F32 = mybir.dt.float32
BF16 = mybir.dt.bfloat16
AF = mybir.ActivationFunctionType
ALU = mybir.AluOpType

N_CORES = 8
B = 4096
DIM_X = 128
DIM_H = 128
HIDDEN = 512
R = B // N_CORES          # rows per core
HR = R // 2               # rows per half
NCH = HIDDEN // 128       # hidden chunks

_NC_CACHE = None


def _build():
    nc = _FastBacc("TRN2", target_bir_lowering=False, debug=False,
                   enable_asserts=False, monotonic_sem_count=0)

    # inW1 = [w1x | w1h]                          (weights for layer 1)
    # inX  = [xa | ha | hna | xb | hb | hnb]      (activations, two row-halves)
    # inW2 = [gs*w2 | (1-gs)*w2 | -gs*c | -(1-gs)*c]
    # aux cols: 0-3 b1' chunks, 4 b2, 5 d0
    inW1 = nc.dram_tensor("inW1", [128, 2 * HIDDEN], BF16, kind="ExternalInput")
    inX = nc.dram_tensor("inX", [128, 6 * HR], BF16, kind="ExternalInput")
    inW2 = nc.dram_tensor("inW2", [128, 2 * NCH * DIM_X + 2 * NCH], BF16,
                          kind="ExternalInput")
    aux = nc.dram_tensor("aux", [128, 8], F32, kind="ExternalInput")

    VO = nc.dram_tensor("VO", [DIM_X, R], BF16, kind="ExternalOutput")
    DO = nc.dram_tensor("DO", [1, R], BF16, kind="ExternalOutput")

    with _TrimTileContext(nc) as tc:
        with tc.tile_pool(name="cst", bufs=1) as cst, \
             tc.tile_pool(name="act", bufs=3) as actp, \
             tc.tile_pool(name="out", bufs=1) as outp, \
             tc.tile_pool(name="psg", bufs=3, space="PSUM") as psg, \
             tc.tile_pool(name="psv", bufs=1, space="PSUM") as psv:
            # PE prewarm: dummy f32 matmuls on a zeroed tile keep the PE-HAM
            # activity window busy during the input DMAs, so real matmuls run
            # at 2.4 GHz instead of 1.2 GHz. (fp32 LOW_HIGH = 2 HW passes per
            # matmul, so 4 python-level matmuls = ~5us of PE busy.)
            wrm = cst.tile([128, 256], F32)
            nc.gpsimd.memset(wrm[:], 0.0)
            pwarm = psv.tile([128, R], F32, tag="pv")
            for _ in range(4):
                nc.tensor.matmul(pwarm[:, 0:256], wrm[:, 0:128], wrm[:],
                                 start=True, stop=True, skip_group_check=True)

            # input DMAs: scalar ring carries the L1 weights (first gate),
            # sync ring carries activations (half a, then half b) then the
            # L2 weights - FIFO order matches need order. aux via SWDGE.
            a1t = cst.tile([128, 2 * HIDDEN], BF16)
            nc.scalar.dma_start(out=a1t[:], in_=inW1[:])
            xt = cst.tile([128, 6 * HR], BF16)
            nc.sync.dma_start(out=xt[:, 0:3 * HR], in_=inX[:, 0:3 * HR])
            nc.sync.dma_start(out=xt[:, 3 * HR:6 * HR], in_=inX[:, 3 * HR:6 * HR])
            w2t = cst.tile([128, 2 * NCH * DIM_X + 2 * NCH], BF16)
            nc.sync.dma_start(out=w2t[:], in_=inW2[:])
            auxt = cst.tile([128, 8], F32)
            nc.gpsimd.dma_start(out=auxt[:], in_=aux[:])

            xa = xt[:, 0 * HR:1 * HR]
            ha = xt[:, 1 * HR:2 * HR]
            hna = xt[:, 2 * HR:3 * HR]
            xb = xt[:, 3 * HR:4 * HR]
            hb = xt[:, 4 * HR:5 * HR]
            hnb = xt[:, 5 * HR:6 * HR]

            def w1x(c):
                return a1t[:, 128 * c:128 * (c + 1)]

            def w1h(c):
                return a1t[:, HIDDEN + 128 * c:HIDDEN + 128 * (c + 1)]

            def w2c(c, br):
                off = br * NCH * DIM_X
                return w2t[:, off + 128 * c:off + 128 * (c + 1)]

            def cmc(c, br):
                off = 2 * NCH * DIM_X + br * NCH
                return w2t[:, off + c:off + c + 1]

            # v and div accumulators (both guidance branches pre-scaled into
            # the weights, so the PSUM sums ARE the combined results)
            pv = psv.tile([128, R], F32, tag="pv")
            pd = psv.tile([1, R], F32, tag="pd")

            # per-chunk L1 groups: [ha | na | hb | nb] over a 2-bank PSUM
            # tile; one start=True per bank, everything else rides the
            # pending-zero bits (overwrite first touch, accumulate after).
            gt = [None] * NCH
            ut = [None] * NCH
            u2t = [None] * NCH

            def l1(c):
                g = psg.tile([128, 4 * HR], F32, tag="g")
                gt[c] = g
                # bank A = cols 0:512 (half a), bank B = cols 512:1024
                nc.tensor.matmul(g[:, 0 * HR:1 * HR], w1x(c), xa, start=True, stop=False)
                nc.tensor.matmul(g[:, 1 * HR:2 * HR], w1x(c), xa, start=False, stop=False)
                nc.tensor.matmul(g[:, 0 * HR:1 * HR], w1h(c), ha, start=False, stop=False)
                nc.tensor.matmul(g[:, 1 * HR:2 * HR], w1h(c), hna, start=False, stop=True)
                nc.tensor.matmul(g[:, 2 * HR:3 * HR], w1x(c), xb, start=True, stop=False)
                nc.tensor.matmul(g[:, 3 * HR:4 * HR], w1x(c), xb, start=False, stop=False)
                nc.tensor.matmul(g[:, 2 * HR:3 * HR], w1h(c), hb, start=False, stop=False)
                nc.tensor.matmul(g[:, 3 * HR:4 * HR], w1h(c), hnb, start=False, stop=True)

            def act(c):
                u = actp.tile([128, 4 * HR], BF16, tag="u")
                ut[c] = u
                nc.scalar.activation(u[:], gt[c][:], AF.Tanh,
                                     bias=auxt[:, c:c + 1], scale=1.0)
                u2 = actp.tile([128, 4 * HR], BF16, tag="u2")
                u2t[c] = u2
                nc.vector.tensor_tensor(u2[:], u[:], u[:], op=ALU.mult)

            def l2(c, first, last):
                u = ut[c]
                # h-branch: halves a,b ; null-branch: halves a,b
                nc.tensor.matmul(pv[:, 0:HR], w2c(c, 0), u[:, 0 * HR:1 * HR],
                                 start=first, stop=False)
                nc.tensor.matmul(pv[:, HR:R], w2c(c, 0), u[:, 2 * HR:3 * HR],
                                 start=False, stop=False)
                nc.tensor.matmul(pv[:, 0:HR], w2c(c, 1), u[:, 1 * HR:2 * HR],
                                 start=False, stop=False)
                nc.tensor.matmul(pv[:, HR:R], w2c(c, 1), u[:, 3 * HR:4 * HR],
                                 start=False, stop=last)

            def pdiv(c, first, last):
                u2 = u2t[c]
                nc.tensor.matmul(pd[0:1, 0:HR], cmc(c, 0), u2[:, 0 * HR:1 * HR],
                                 start=first, stop=False)
                nc.tensor.matmul(pd[0:1, HR:R], cmc(c, 0), u2[:, 2 * HR:3 * HR],
                                 start=False, stop=False)
                nc.tensor.matmul(pd[0:1, 0:HR], cmc(c, 1), u2[:, 1 * HR:2 * HR],
                                 start=False, stop=False)
                nc.tensor.matmul(pd[0:1, HR:R], cmc(c, 1), u2[:, 3 * HR:4 * HR],
                                 start=False, stop=last)

            # PE emission order keeps the queue from blocking on tanh: run
            # L1 of later chunks while ACT/DVE chew on earlier ones.
            l1(0)
            act(0)
            l1(1)
            act(1)
            l1(2)
            act(2)
            l2(0, True, False)
            l1(3)
            act(3)
            pdiv(0, True, False)
            l2(1, False, False)
            pdiv(1, False, False)
            l2(2, False, False)
            pdiv(2, False, False)
            l2(3, False, True)
            pdiv(3, False, True)

            # evacuate: bias-add on DVE (ACT may still be on the last tanh),
            # outputs in bf16, VO on sync ring / DO on scalar ring so the
            # two completion receipts overlap.
            vout = outp.tile([128, R], BF16)
            nc.vector.tensor_scalar(vout[:], pv[:], auxt[:, 4:5], None, op0=ALU.add)
            nc.sync.dma_start(out=VO[:], in_=vout[:])
            dout = outp.tile([1, R], BF16)
            nc.vector.tensor_scalar(dout[:], pd[0:1, :], auxt[0:1, 5:6], None,
                                    op0=ALU.add)
            nc.scalar.dma_start(out=DO[:], in_=dout[:])
    nc.compile()
    return nc


def _get_nc():
    global _NC_CACHE
    if _NC_CACHE is None:
        _NC_CACHE = _build()
    return _NC_CACHE


def _prep_in_maps(state, h, h_null, t, guidance_scale, W1, b1, W2, b2):
    f32 = np.float32
    bf = ml_dtypes.bfloat16
    xTf = state[:, :DIM_X].T.astype(bf)                            # (128, B)
    hTf = h.T.astype(bf)
    hnTf = h_null.T.astype(bf)
    w1f = np.ascontiguousarray(
        np.concatenate([W1[:DIM_X], W1[DIM_X:DIM_X + DIM_H]], axis=1).astype(bf))
    b1p = (b1.astype(f32) + t.astype(f32)[0] * W1[DIM_X + DIM_H].astype(f32))
    w2r = W2.astype(f32).reshape(NCH, 128, DIM_X).transpose(1, 0, 2).reshape(128, NCH * DIM_X)
    cvec = (W1[:DIM_X].astype(np.float64) * W2.astype(np.float64).T).sum(0)  # (512,)
    d0 = cvec.sum()
    cmatf = cvec.reshape(NCH, 128).T.astype(f32)                   # (128, NCH)
    gs = float(guidance_scale.astype(f32)[0])
    w2cf = np.ascontiguousarray(
        np.concatenate([gs * w2r, (1.0 - gs) * w2r,
                        -gs * cmatf, -(1.0 - gs) * cmatf], axis=1).astype(bf))

    auxf = np.zeros((128, 8), f32)
    auxf[:, 0:4] = b1p.reshape(NCH, 128).T
    auxf[:, 4] = b2.astype(f32)
    auxf[:, 5] = d0

    in_maps = []
    for i in range(N_CORES):
        sl_a = slice(i * R, i * R + HR)
        sl_b = slice(i * R + HR, (i + 1) * R)
        in_maps.append({
            "inW1": w1f,
            "inX": np.ascontiguousarray(
                np.concatenate([xTf[:, sl_a], hTf[:, sl_a], hnTf[:, sl_a],
                                xTf[:, sl_b], hTf[:, sl_b], hnTf[:, sl_b]],
                               axis=1)),
            "inW2": w2cf,
            "aux": auxf,
        })
    return in_maps


def kernel(state, h, h_null, t, guidance_scale, W1, b1, W2, b2, _trace=False):
    nc = _get_nc()
    in_maps = _prep_in_maps(state, h, h_null, t, guidance_scale, W1, b1, W2, b2)
    res = run_bass_kernel_spmd(nc, in_maps, list(range(N_CORES)), trace=_trace)
    out = np.empty((B, DIM_X + 1), np.float32)
    for i in range(N_CORES):
        sl = slice(i * R, (i + 1) * R)
        out[sl, :DIM_X] = res.results[i]["VO"].astype(np.float32).T
        out[sl, DIM_X] = res.results[i]["DO"][0].astype(np.float32)
    if _trace:
        return out, res
    return out


# revision 4
# speedup vs baseline: 1.1103x; 1.0630x over previous
"""Trainium2 Bass kernel for CombinedVectorField (CFG vector field + exact
Jacobian-trace divergence).

Math: with u = tanh(x@W1x + h@W1h + b1'), b1' = b1 + t*W1[256],
  v(x,h)  = u @ W2 + b2
  div(x,h)= sum_k (1-u_k^2) c_k = d0 - (u*u) @ c,   c_k = sum_i W1x[i,k] W2[k,i]
Output = concat[(1-gs)*v_null + gs*v_h, (1-gs)*div_null + gs*div_h].

Sharding: pure data parallel - each of the 8 cores takes 512 batch rows
(both guidance branches), weights replicated, feature-major layouts.

Schedule notes:
- Input blobs are packed by NEED TIME, not semantics: blob A carries the
  first row-half activations plus chunk-0 weights so L1 starts ~1.2us
  earlier; remaining weights / second half / W2 stream in behind it.
- Each (chunk, half) L1 output is one [128,512] PSUM bank holding both
  guidance branches side by side -> one tanh ACTIVATE per group with the
  per-partition chunk bias (single start=True per bank; later partial-
  region writes ride the pending-zero overwrite-then-accumulate bits).
- The divergence matmuls are column-tiled 4-way (tile_position=(0,32c))
  so the four chunk partials run concurrently, land in distinct
  partitions of one bank, and ship to the host as 4 bf16 rows that the
  host sums with d0. They sit after the v matmuls so they are off the
  VO critical path.
- PE prewarm reads uninitialized SBUF (values irrelevant) so it needs no
  memset and starts right at kernel entry, pulling the HAM warm-clock
  transition earlier.
"""
import sys

sys.path.insert(0, "/opt/trn_rl_repo")

import ml_dtypes
import numpy as np

import concourse.bass as bass
import concourse.tile as tile
from concourse import bacc, mybir
from concourse.bass_utils import run_bass_kernel_spmd
from concourse.vector_clock import ScopedClock


class _TrimTileContext(tile.TileContext):
    """TileContext with the final all-engine barrier dropped from the
    teardown and the mid barrier reduced to sem-only (no per-engine
    drains). The head drain still waits for every semaphore (incl.
    output-DMA completion) and semaphores are still cleared for the next
    execution; only the trailing barrier (nothing executes after it) is
    elided."""

    def _drain_and_barrier(self, tick_clock, wait_clock):
        drain_inst = self.nc.sync.drain()
        wait_clock.add_sem_waits(
            drain_inst.ins, ScopedClock({None: tick_clock.global_clock})
        )
        self.nc.all_engine_barrier(sem_only=True)
        popped = self.nc._tile_sem_poison_stack.pop()
        assert popped is self._sem_poison
        self.nc.clear_and_free_semaphores(list(self.sems.allocated().values()))


class _FastBacc(bacc.Bacc):
    """Bacc whose constructor-time all-engine barrier (after the const-tile
    memsets) is sem-only - the per-engine drains there cost ~1us of kernel
    head time and order nothing we rely on beyond the memsets, which the
    event-semaphore barrier already orders."""

    def all_engine_barrier(self, *, sem_only: bool = False):
        super().all_engine_barrier(sem_only=True)

F32 = mybir.dt.float32
BF16 = mybir.dt.bfloat16
AF = mybir.ActivationFunctionType
ALU = mybir.AluOpType

N_CORES = 8
B = 4096
DIM_X = 128
DIM_H = 128
HIDDEN = 512
R = B // N_CORES          # rows per core
HR = R // 2               # rows per half
NCH = HIDDEN // 128       # hidden chunks

_NC_CACHE = None


def _build():
    nc = _FastBacc("TRN2", target_bir_lowering=False, debug=False,
                   enable_asserts=False, monotonic_sem_count=0)

    # inA = [xa | ha | hna | w1x_c0 | w1h_c0]   (first gate, scalar ring)
    # inB = [w1x_c123 | w1h_c123]               (sync ring)
    # inC = [xb | hb | hnb]                     (scalar ring, 2nd)
    # inD = [gs*w2 | (1-gs)*w2 | -gs*c | -(1-gs)*c | mask]  (gpsimd)
    # aux cols (f32): 0-3 b1' chunks, 4 b2
    inA = nc.dram_tensor("inA", [128, 4 * HR], BF16, kind="ExternalInput")
    inB = nc.dram_tensor("inB", [128, 6 * 128], BF16, kind="ExternalInput")
    inC = nc.dram_tensor("inC", [128, 3 * HR], BF16, kind="ExternalInput")
    inD = nc.dram_tensor("inD", [128, 2 * NCH * DIM_X + 2 * NCH + 1], BF16,
                         kind="ExternalInput")
    aux = nc.dram_tensor("aux", [128, 8], F32, kind="ExternalInput")

    VO = nc.dram_tensor("VO", [DIM_X, R], BF16, kind="ExternalOutput")
    DO = nc.dram_tensor("DO", [NCH, R], BF16, kind="ExternalOutput")

    with _TrimTileContext(nc) as tc:
        with tc.tile_pool(name="cst", bufs=1) as cst, \
             tc.tile_pool(name="act", bufs=8) as actp, \
             tc.tile_pool(name="out", bufs=1) as outp, \
             tc.tile_pool(name="psg", bufs=4, space="PSUM") as psg, \
             tc.tile_pool(name="psv", bufs=1, space="PSUM") as psv:
            # PE prewarm: keeps the PE-HAM activity window busy from kernel
            # entry so real matmuls hit 2.4 GHz sooner. The memset rides the
            # vector engine (idle at body start, ~0.3us earlier than gpsimd
            # which still owes the framework const-tile memsets).
            wrm = cst.tile([128, R], BF16)
            nc.vector.memset(wrm[:], 0.0)
            pwarm = psv.tile([128, R], F32, tag="pd")
            for _ in range(12):
                nc.tensor.matmul(pwarm[:], wrm[:, 0:128], wrm[:],
                                 start=True, stop=True, skip_group_check=True)

            at = cst.tile([128, 4 * HR], BF16)
            nc.scalar.dma_start(out=at[:], in_=inA[:])
            ct = cst.tile([128, 3 * HR], BF16)
            nc.scalar.dma_start(out=ct[:], in_=inC[:])
            bt = cst.tile([128, 6 * 128], BF16)
            nc.sync.dma_start(out=bt[:], in_=inB[:])
            auxt = cst.tile([128, 8], F32)
            nc.gpsimd.dma_start(out=auxt[:], in_=aux[:])
            dt = cst.tile([128, 2 * NCH * DIM_X + 2 * NCH + 1], BF16)
            nc.gpsimd.dma_start(out=dt[:], in_=inD[:])

            xa = at[:, 0 * HR:1 * HR]
            ha = at[:, 1 * HR:2 * HR]
            hna = at[:, 2 * HR:3 * HR]
            xb = ct[:, 0 * HR:1 * HR]
            hb = ct[:, 1 * HR:2 * HR]
            hnb = ct[:, 2 * HR:3 * HR]

            def w1x(c):
                if c == 0:
                    return at[:, 3 * HR:3 * HR + 128]
                return bt[:, 128 * (c - 1):128 * c]

            def w1h(c):
                if c == 0:
                    return at[:, 3 * HR + 128:3 * HR + 256]
                return bt[:, 384 + 128 * (c - 1):384 + 128 * c]

            def w2c(c, br):
                off = br * NCH * DIM_X
                return dt[:, off + 128 * c:off + 128 * (c + 1)]

            def cmc(c, br):
                off = 2 * NCH * DIM_X + br * NCH
                return dt[:, off + c:off + c + 1]

            pva = psv.tile([128, HR], F32, tag="pva")
            pvb = psv.tile([128, HR], F32, tag="pvb")
            pd = psv.tile([128, R], F32, tag="pd")

            ut = {}
            u2t = {}
            gt = {}

            def l1(c, half, x_, h_, hn_):
                g = psg.tile([128, 2 * HR], F32, tag="g")
                gt[(c, half)] = g
                nc.tensor.matmul(g[:, 0:HR], w1x(c), x_, start=True, stop=False)
                nc.tensor.matmul(g[:, HR:2 * HR], w1x(c), x_, start=False, stop=False)
                nc.tensor.matmul(g[:, 0:HR], w1h(c), h_, start=False, stop=False)
                nc.tensor.matmul(g[:, HR:2 * HR], w1h(c), hn_, start=False, stop=True)

            def act(c, half):
                u = actp.tile([128, 2 * HR], BF16, tag="u")
                ut[(c, half)] = u
                nc.scalar.activation(u[:], gt[(c, half)][:], AF.Tanh,
                                     bias=auxt[:, c:c + 1], scale=1.0)
                u2 = actp.tile([128, 2 * HR], BF16, tag="u2")
                u2t[(c, half)] = u2
                nc.vector.tensor_tensor(u2[:], u[:], u[:], op=ALU.mult)

            def l2(c, half, pv, first, last):
                u = ut[(c, half)]
                nc.tensor.matmul(pv[:], w2c(c, 0), u[:, 0:HR],
                                 start=first, stop=False)
                nc.tensor.matmul(pv[:], w2c(c, 1), u[:, HR:2 * HR],
                                 start=False, stop=last)

            def pdiv(br, half, first, last):
                # 4 chunk partials run concurrently on distinct col groups,
                # landing at partitions {0,32,64,96} of the pd bank.
                cs = slice(0, HR) if half == 0 else slice(HR, R)
                us = slice(0, HR) if br == 0 else slice(HR, 2 * HR)
                for c in range(NCH):
                    nc.tensor.matmul(pd[32 * c:32 * c + 1, cs], cmc(c, br),
                                     u2t[(c, half)][:, us],
                                     start=(first and c == 0),
                                     stop=(last and c == NCH - 1),
                                     tile_position=(0, 32 * c))

            # a-half L1 + activations stream first (gated on inA/inB only)
            for c in range(NCH):
                l1(c, 0, xa, ha, hna)
                act(c, 0)
            for c in range(NCH):
                l1(c, 1, xb, hb, hnb)
                act(c, 1)
            for c in range(NCH):
                l2(c, 0, pva, c == 0, c == NCH - 1)
            pdiv(0, 0, True, False)
            pdiv(1, 0, False, False)
            for c in range(NCH):
                l2(c, 1, pvb, c == 0, c == NCH - 1)
            pdiv(0, 1, False, False)
            pdiv(1, 1, False, True)

            # evacuations on DVE (ACT is still busy with the tanh chain):
            # v halves as soon as their accumulator closes, divergence
            # partials as one bank copy -> 4-row DMA, summed on the host.
            vouta = outp.tile([128, HR], BF16)
            nc.vector.tensor_scalar(vouta[:], pva[:], auxt[:, 4:5], None, op0=ALU.add)
            nc.sync.dma_start(out=VO[:, 0:HR], in_=vouta[:])
            voutb = outp.tile([128, HR], BF16)
            nc.vector.tensor_scalar(voutb[:], pvb[:], auxt[:, 4:5], None, op0=ALU.add)
            nc.sync.dma_start(out=VO[:, HR:R], in_=voutb[:])
            pdc = outp.tile([128, R], BF16)
            nc.vector.tensor_copy(pdc[:], pd[:])
            nc.scalar.dma_start(out=DO[:], in_=pdc[0:97:32, :])
    nc.compile()
    return nc


def _get_nc():
    global _NC_CACHE
    if _NC_CACHE is None:
        _NC_CACHE = _build()
    return _NC_CACHE


def _prep_in_maps(state, h, h_null, t, guidance_scale, W1, b1, W2, b2):
    f32 = np.float32
    bf = ml_dtypes.bfloat16
    xTf = state[:, :DIM_X].T.astype(bf)                            # (128, B)
    hTf = h.T.astype(bf)
    hnTf = h_null.T.astype(bf)
    w1xf = W1[:DIM_X].astype(bf)                                   # (128, 512)
    w1hf = W1[DIM_X:DIM_X + DIM_H].astype(bf)
    b1p = (b1.astype(f32) + t.astype(f32)[0] * W1[DIM_X + DIM_H].astype(f32))
    w2r = W2.astype(f32).reshape(NCH, 128, DIM_X).transpose(1, 0, 2).reshape(128, NCH * DIM_X)
    cvec = (W1[:DIM_X].astype(np.float64) * W2.astype(np.float64).T).sum(0)  # (512,)
    d0 = float(cvec.sum())
    cmatf = cvec.reshape(NCH, 128).T.astype(f32)                   # (128, NCH)
    gs = float(guidance_scale.astype(f32)[0])
    mask = np.zeros((128, 1), f32)
    mask[::32] = 1.0  # unused on-device now; kept for layout stability
    inD = np.ascontiguousarray(
        np.concatenate([gs * w2r, (1.0 - gs) * w2r,
                        -gs * cmatf, -(1.0 - gs) * cmatf, mask], axis=1).astype(bf))
    inB = np.ascontiguousarray(
        np.concatenate([w1xf[:, 128:], w1hf[:, 128:]], axis=1))

    auxf = np.zeros((128, 8), f32)
    auxf[:, 0:4] = b1p.reshape(NCH, 128).T
    auxf[:, 4] = b2.astype(f32)

    in_maps = []
    for i in range(N_CORES):
        sl_a = slice(i * R, i * R + HR)
        sl_b = slice(i * R + HR, (i + 1) * R)
        in_maps.append({
            "inA": np.ascontiguousarray(
                np.concatenate([xTf[:, sl_a], hTf[:, sl_a], hnTf[:, sl_a],
                                w1xf[:, 0:128], w1hf[:, 0:128]], axis=1)),
            "inB": inB,
            "inC": np.ascontiguousarray(
                np.concatenate([xTf[:, sl_b], hTf[:, sl_b], hnTf[:, sl_b]],
                               axis=1)),
            "inD": inD,
            "aux": auxf,
        })
    return in_maps, d0


def kernel(state, h, h_null, t, guidance_scale, W1, b1, W2, b2, _trace=False):
    nc = _get_nc()
    in_maps, d0 = _prep_in_maps(state, h, h_null, t, guidance_scale,
                                W1, b1, W2, b2)
    res = run_bass_kernel_spmd(nc, in_maps, list(range(N_CORES)), trace=_trace)
    out = np.empty((B, DIM_X + 1), np.float32)
    for i in range(N_CORES):
        sl = slice(i * R, (i + 1) * R)
        out[sl, :DIM_X] = res.results[i]["VO"].astype(np.float32).T
        out[sl, DIM_X] = res.results[i]["DO"].astype(np.float32).sum(0) + d0
    if _trace:
        return out, res
    return out
